# revision 31
# baseline (speedup 1.0000x reference)
"""Trainium2 Bass kernel: GroupNorm + single-head self-attention block.

Reference computation (per batch b, x: [C=512, HW=1024] after flattening spatial):
    xn   = groupnorm(x, 8 groups over C, eps=1e-5) * gamma + beta
    qkv  = qkv_w @ xn + qkv_b            # [3C, HW]
    s    = q^T k * C^-0.5
    out  = x + out_w @ (v @ softmax(s)) + out_b

Implementation strategy (fp8 DoubleRow everywhere):
  - Host folds G = Wq^T Wk and H = Wo Wv, so per batch the kernel runs
        t1  = G^T xn                  [C, HW]   (q/k projections fused)
        v'T = xn^T H^T                [HW, C]   (v and out projections fused)
        sT  = xn^T t1                 [HW, HW]  (= k^T q, transposed scores)
        e   = exp(sT * C^-0.5 - 2)              (shift cancels in normalization)
        cs  = ones^T e  (colsum), recip = 1/cs
        res = v'^ e                   [C, HW]
        out = x + res * recip + outb_eff
    This removes 1/3 of the matmul MACs and their PSUM->SBUF copies.
  - All big matmuls run as float8e4 (TRN E4M3, +-240) with
    MatmulPerfMode.DoubleRow: K=256 per instruction at the same wall cost as a
    K=128 fp32r matmul (measured 244ns vs 254ns for N=512) -> 2.08x PE rate.
    Weights are pre-scaled x16 on host so their sigma~0.044 lands in e4m3's
    normal range; the 1/16 is folded into the post-matmul activation scale.
    Measured end-to-end rel err ~9e-3 vs the 2e-2 gate (numpy-sim predicted
    8.6e-3).
  - qkv biases are handled exactly: the bq-side rank-1 term folds into t1's
    activation bias (Wk^T bq); the per-pixel-i terms scale whole columns of e
    and cancel in the colsum division; v/out biases fold into outb_eff.

Sharding: data-parallel over batch, 32 batches / 8 cores = 4 per core.
"""

import json
import os

import numpy as np
import ml_dtypes

import concourse.bass as bass
import concourse.mybir as mybir
import concourse.tile as tile
from concourse.bass_utils import run_bass_kernel_spmd


def _spill_multiwaits(raw: bytes) -> bytes:
    """Walrus in this toolchain accepts only one sync-wait command per
    instruction descriptor. Spill extra on_wait entries onto single-wait
    EventSemaphore instructions inserted immediately before, on the same
    engine queue (the exact pattern Tile's own barriers use), which is
    semantically identical: the queue blocks at the same point either way.
    """
    j = json.loads(raw)
    n = 0
    for fn in j.get("functions", []):
        for blk in fn.get("blocks", []):
            out = []
            for inst in blk.get("instructions", []):
                si = inst.get("sync_info") or {}
                waits = si.get("on_wait") or []
                if len(waits) > 1 and inst.get("engine"):
                    for spilled in waits[:-1]:
                        n += 1
                        out.append({
                            "debug": inst.get("debug", 0),
                            "engine": inst["engine"],
                            "ins": [],
                            "name": f"{inst['name']}-sw{n}",
                            "opcode": "EventSemaphore",
                            "outs": [],
                            "sync_info": {"on_update": [], "on_wait": [spilled]},
                        })
                    si["on_wait"] = waits[-1:]
                out.append(inst)
            blk["instructions"] = out
    return json.dumps(j).encode()


_orig_to_json_bytes = bass.Bass.to_json_bytes


def _patched_to_json_bytes(self):
    return _spill_multiwaits(_orig_to_json_bytes(self))


bass.Bass.to_json_bytes = _patched_to_json_bytes

F32 = mybir.dt.float32
F32R = mybir.dt.float32r  # stats-path matmuls only
FP8 = mybir.dt.float8e4
DR = mybir.MatmulPerfMode.DoubleRow
NP8 = ml_dtypes.float8_e4m3  # TRN e4m3: max normal +-240

N_CORES = 8
B_TOTAL = 32
B_PER_CORE = B_TOTAL // N_CORES
C = 512
HW = 1024
GROUPS = 8
EPS = 1e-5
SCALE = float(C) ** -0.5
EXP_SHIFT = -2.0     # cancels in colsum division; keeps e < 34 << 240
WSCALE = 16.0        # weight pre-scale into e4m3 normal range

CT = C // 128   # 4 channel tiles
PT = HW // 128  # 8 pixel tiles
NB = HW // 512  # 2 free-dim blocks of 512
MUL = mybir.AluOpType.mult
ADD = mybir.AluOpType.add


def build_nc():
    nc = bass.Bass()

    x_d = nc.dram_tensor("x", [B_PER_CORE, C, HW], F32, kind="ExternalInput")
    g_d = nc.dram_tensor("g8", [C, C], FP8, kind="ExternalInput")     # 16*Wq^T Wk  [c_in, c_out]
    h_d = nc.dram_tensor("h8", [C, C], FP8, kind="ExternalInput")     # 16*(Wo Wv)^T [c_in, c_out]
    t1b_d = nc.dram_tensor("t1b", [C], F32, kind="ExternalInput")     # Wk^T bq
    outb_d = nc.dram_tensor("outb", [C], F32, kind="ExternalInput")   # out_b + Wo bv
    gamma_d = nc.dram_tensor("gamma", [C], F32, kind="ExternalInput")
    beta_d = nc.dram_tensor("beta", [C], F32, kind="ExternalInput")
    sel_d = nc.dram_tensor("sel", [C, GROUPS], F32, kind="ExternalInput")
    selT_d = nc.dram_tensor("selT", [GROUPS, C], F32, kind="ExternalInput")
    out_d = nc.dram_tensor("out", [B_PER_CORE, C, HW], F32, kind="ExternalOutput")
    warmdump_d = nc.dram_tensor("warmdump", [128, 4], F32)

    with tile.TileContext(nc) as tc:
        with (
            tc.tile_pool(name="wpool", bufs=1) as wpool,
            tc.tile_pool(name="xpool", bufs=3) as xpool,
            tc.tile_pool(name="xnpool", bufs=2) as xnpool,
            tc.tile_pool(name="t1pool", bufs=1) as t1pool,
            tc.tile_pool(name="vppool", bufs=1) as vppool,
            tc.tile_pool(name="expool", bufs=1) as expool,
            tc.tile_pool(name="rpool", bufs=2) as rpool,
            tc.tile_pool(name="respool", bufs=1) as respool,
            tc.tile_pool(name="ftpool", bufs=2) as ftpool,
            tc.tile_pool(name="spool", bufs=2) as spool,
            tc.tile_pool(name="wps", bufs=3, space=bass.MemorySpace.PSUM) as wps,
            tc.tile_pool(name="stps", bufs=1, space=bass.MemorySpace.PSUM) as stps,
        ):
            xts = {}

            def load_x(bb):
                xt = xpool.tile([128, CT, HW], F32, tag="xt")
                xts[bb] = xt
                # per-c-tile chunks so bn_stats can start before the full
                # load; issued from the otherwise-idle gpsimd queue so they
                # don't serialize behind the output stores on the sync queue
                for t in range(CT):
                    nc.gpsimd.dma_start(
                        out=xt[:, t],
                        in_=x_d[bb, t * 128:(t + 1) * 128, :])
                return xt

            # x(0) first: its consumer chain (stats -> xn -> t1) is the
            # critical path to the first big matmul
            load_x(0)

            # ---- tiny constants (cheap DMAs / memsets) ----
            eps_sb = wpool.tile([128, 1], F32)
            nc.vector.memset(eps_sb, EPS)
            shift_sb = wpool.tile([128, 1], F32)
            nc.vector.memset(shift_sb, EXP_SHIFT)
            ones_st = wpool.tile([128, 256], F32)
            nc.vector.memset(ones_st, 1.0)
            ones8 = wpool.tile([128, 2, 128], FP8)
            nc.vector.tensor_copy(ones8, ones_st.rearrange("p (a b) -> p a b", a=2))
            # HAM warmup: keep the PE busy under the startup DMA window so the
            # clock gate reaches 8/8 (2.4GHz) before the first real matmul.
            warm_st = wpool.tile([128, 512], F32)
            nc.vector.memset(warm_st, 0.0)
            warm_rhs = wpool.tile([128, 512], F32R)
            nc.vector.tensor_copy(warm_rhs, warm_st)
            ones_r = wpool.tile([128, 128], F32R)
            nc.vector.tensor_copy(ones_r, ones_st[:, 0:128])
            warm_ps = stps.tile([128, 512], F32, tag="gps")
            for w in range(50):
                nc.tensor.matmul(warm_ps, lhsT=ones_r, rhs=warm_rhs,
                                 start=True, stop=True)
            warm_out = wpool.tile([128, 4], F32)
            nc.vector.tensor_copy(warm_out, warm_ps[:, 0:4])
            nc.sync.dma_start(out=warmdump_d[:, :], in_=warm_out)

            sel_st = wpool.tile([128, CT, GROUPS], F32)
            nc.sync.dma_start(out=sel_st, in_=sel_d.rearrange("(t p) g -> p t g", p=128))
            sel_sb = wpool.tile([128, CT, GROUPS], F32R)
            nc.vector.tensor_copy(sel_sb, sel_st)
            selT_st = wpool.tile([GROUPS, C], F32)
            nc.sync.dma_start(out=selT_st, in_=selT_d[:, :])
            selT_sb = wpool.tile([GROUPS, C], F32R)
            nc.vector.tensor_copy(selT_sb, selT_st)
            t1b_sb = wpool.tile([128, CT], F32)
            nc.sync.dma_start(out=t1b_sb, in_=t1b_d.rearrange("(m p) -> p m", p=128))
            outb_sb = wpool.tile([128, CT], F32)
            nc.sync.dma_start(out=outb_sb, in_=outb_d.rearrange("(m p) -> p m", p=128))
            gamma_sb = wpool.tile([128, CT], F32)
            nc.sync.dma_start(out=gamma_sb, in_=gamma_d.rearrange("(m p) -> p m", p=128))
            beta_sb = wpool.tile([128, CT], F32)
            nc.sync.dma_start(out=beta_sb, in_=beta_d.rearrange("(m p) -> p m", p=128))

            # ---- fused weights (fp8, pre-scaled x16 on host) ----
            g_sb = wpool.tile([128, CT, C], FP8)
            h_sb = wpool.tile([128, CT, C], FP8)
            g_r = g_d.rearrange("(t p) o -> p t o", p=128)
            h_r = h_d.rearrange("(t p) o -> p t o", p=128)
            for t in range(CT):
                nc.sync.dma_start(out=g_sb[:, t], in_=g_r[:, t])
                nc.sync.dma_start(out=h_sb[:, t], in_=h_r[:, t])

            def norm_stats(bb):
                """GroupNorm stats for batch bb -> per-channel (scale, shift)."""
                xt = xts[bb]
                stats3 = spool.tile([128, CT, 4], F32, tag="stats3")
                nc.vector.memset(stats3, 0.0)
                for t in range(CT):
                    st6 = spool.tile([128, 2, 6], F32, tag="st6")
                    for sg in range(2):
                        nc.vector.bn_stats(out=st6[:, sg], in_=xt[:, t, sg * 512:(sg + 1) * 512])
                    nc.vector.bn_aggr(out=stats3[:, t, 0:2], in_=st6)
                    nc.vector.tensor_mul(stats3[:, t, 2:3], stats3[:, t, 0:1], stats3[:, t, 0:1])
                stats3r = spool.tile([128, CT, 4], F32R, tag="stats3r")
                nc.vector.tensor_copy(stats3r, stats3)
                gps = stps.tile([GROUPS, 4], F32, tag="gps")
                for t in range(CT):
                    nc.tensor.matmul(gps, lhsT=sel_sb[:, t], rhs=stats3r[:, t],
                                     start=(t == 0), stop=(t == CT - 1))
                # group var = E[var_c] + E[mean_c^2] - E[mean_c]^2 ; then rstd
                gsb = spool.tile([GROUPS, 4], F32, tag="gsb")
                nc.vector.tensor_copy(gsb, gps)
                gs = spool.tile([GROUPS, 4], F32, tag="gs")
                nc.vector.memset(gs, 0.0)
                tmp8 = spool.tile([GROUPS, 1], F32, tag="tmp8")
                nc.vector.tensor_mul(tmp8, gsb[:, 0:1], gsb[:, 0:1])
                nc.vector.tensor_add(gs[:, 1:2], gsb[:, 1:2], gsb[:, 2:3])
                nc.vector.tensor_sub(gs[:, 1:2], gs[:, 1:2], tmp8)
                # rstd = exp(-0.5*ln(var+eps)): Ln/Exp share an ACT table set
                # with the softmax Exp, avoiding 2 x 1.3us table reloads/batch
                # that Sqrt (different set) would trigger.
                nc.scalar.activation(gs[:, 1:2], gs[:, 1:2],
                                     mybir.ActivationFunctionType.Ln,
                                     bias=eps_sb[:GROUPS])
                nc.scalar.activation(gs[:, 1:2], gs[:, 1:2],
                                     mybir.ActivationFunctionType.Exp,
                                     scale=-0.5)
                nc.vector.tensor_copy(gs[:, 0:1], gsb[:, 0:1])
                # broadcast group stats back to channel partitions
                gsr = spool.tile([GROUPS, 4], F32R, tag="gsr")
                nc.vector.tensor_copy(gsr, gs)
                csps = stps.tile([128, CT, 4], F32, tag="csps")
                for t in range(CT):
                    nc.tensor.matmul(csps[:, t], lhsT=selT_sb[:, t * 128:(t + 1) * 128],
                                     rhs=gsr, start=True, stop=True)
                # per-channel affine: xn = x * s + tt
                stv = spool.tile([128, CT, 2], F32, tag="stv")
                for t in range(CT):
                    tmpc = spool.tile([128, 1], F32, tag="tmpc")
                    nc.vector.tensor_mul(stv[:, t, 0:1], csps[:, t, 1:2], gamma_sb[:, t:t + 1])
                    nc.vector.tensor_mul(tmpc, csps[:, t, 0:1], stv[:, t, 0:1])
                    nc.vector.tensor_sub(stv[:, t, 1:2], beta_sb[:, t:t + 1], tmpc)
                return stv

            def norm_apply(bb, stv):
                """xn = x*s + t, quantized straight to fp8 for the matmuls."""
                xt = xts[bb]
                xn = xnpool.tile([128, CT, HW], FP8, tag="xn")
                # all first-halves first: the first t1 accumulation group only
                # reads columns 0:512 of c-tiles 0..1, so it can start early
                for h in range(NB):
                    for t in range(CT):
                        nc.vector.tensor_scalar(
                            out=xn[:, t, h * 512:(h + 1) * 512],
                            in0=xt[:, t, h * 512:(h + 1) * 512],
                            scalar1=stv[:, t, 0:1], scalar2=stv[:, t, 1:2],
                            op0=MUL, op1=ADD)
                return xn

            def part1(bb, xn):
                """t1 = G^T xn and v'T = xn^T H^T, both fp8."""
                t1 = t1pool.tile([128, CT, HW], FP8, tag="t1")
                for m in range(CT):
                    wt = wps.tile([128, 2 * 512], F32, tag="mm")
                    for n in range(NB):
                        for tp in range(2):
                            nc.tensor.matmul(
                                wt[:, n * 512:(n + 1) * 512],
                                lhsT=g_sb[:, 2 * tp:2 * tp + 2, m * 128:(m + 1) * 128],
                                rhs=xn[:, 2 * tp:2 * tp + 2, n * 512:(n + 1) * 512],
                                start=(tp == 0), stop=(tp == 1), perf_mode=DR)
                    nc.scalar.activation(t1[:, m, :], wt,
                                         mybir.ActivationFunctionType.Identity,
                                         bias=t1b_sb[:, m:m + 1], scale=1.0 / WSCALE)
                vpT = vppool.tile([128, PT, C], FP8, tag="vpT")
                for pp in range(PT // 2):
                    wt = wps.tile([128, 2 * 512], F32, tag="mm")
                    for i in range(2):
                        p = 2 * pp + i
                        for tp in range(2):
                            nc.tensor.matmul(
                                wt[:, i * 512:(i + 1) * 512],
                                lhsT=xn[:, 2 * tp:2 * tp + 2, p * 128:(p + 1) * 128],
                                rhs=h_sb[:, 2 * tp:2 * tp + 2, :],
                                start=(tp == 0), stop=(tp == 1), perf_mode=DR)
                    if pp % 2 == 0:
                        nc.scalar.activation(vpT[:, 2 * pp:2 * pp + 2, :],
                                             wt.rearrange("p (a b) -> p a b", a=2),
                                             mybir.ActivationFunctionType.Copy,
                                             scale=1.0 / WSCALE)
                    else:
                        # alternate drains between ACT and DVE: both engines
                        # run ~90% busy in steady state, so split the load
                        nc.vector.tensor_scalar_mul(
                            vpT[:, 2 * pp:2 * pp + 2, :],
                            wt.rearrange("p (a b) -> p a b", a=2),
                            1.0 / WSCALE)
                return t1, vpT

            def part2a(bb, xn, t1):
                """transposed scores -> exp (fp8) -> colsum -> recip.

                """
                expT = expool.tile([128, PT, HW], FP8, tag="expT")
                for jm in range(PT):
                    wt = wps.tile([128, 2 * 512], F32, tag="mm")
                    for n in range(NB):
                        for tp in range(2):
                            nc.tensor.matmul(
                                wt[:, n * 512:(n + 1) * 512],
                                lhsT=xn[:, 2 * tp:2 * tp + 2, jm * 128:(jm + 1) * 128],
                                rhs=t1[:, 2 * tp:2 * tp + 2, n * 512:(n + 1) * 512],
                                start=(tp == 0), stop=(tp == 1), perf_mode=DR)
                    nc.scalar.activation(expT[:, jm, :], wt,
                                         mybir.ActivationFunctionType.Exp,
                                         bias=shift_sb, scale=SCALE)
                colp = wps.tile([128, 2 * 512], F32, tag="mm")
                for n in range(NB):
                    for jp in range(PT // 2):
                        nc.tensor.matmul(
                            colp[:, n * 512:(n + 1) * 512],
                            lhsT=ones8,
                            rhs=expT[:, 2 * jp:2 * jp + 2, n * 512:(n + 1) * 512],
                            start=(jp == 0), stop=(jp == PT // 2 - 1), perf_mode=DR)
                # recip = exp(-ln(colsum)) on ACT: ln and exp share the
                # resident table set, and the exact DVE reciprocal (6.5us for
                # [128,1024]) would gate the av-stage PSUM rotation.
                lncs = rpool.tile([128, HW], F32, tag="lncs")
                nc.scalar.activation(lncs, colp,
                                     mybir.ActivationFunctionType.Ln)
                recip = rpool.tile([128, HW], F32, tag="recip")
                nc.scalar.activation(recip, lncs,
                                     mybir.ActivationFunctionType.Exp,
                                     scale=-1.0)
                return expT, recip

            def part2b(bb, vpT, expT, recip):
                """res = v' e, normalize, bias, residual, store."""
                xt = xts.pop(bb)
                # Drain every av psum to SBUF via a cheap ACT copy right away:
                # the psum slot frees in ~1.1us regardless of where the DVE is
                # in its queue, so the next batch's t1/scores rotation never
                # starves on a psum bank held hostage by a late normalize.
                resT = respool.tile([128, CT, HW], F32, tag="resT")
                for m in range(CT):
                    wt = wps.tile([128, 2 * 512], F32, tag="mm")
                    for n in range(NB):
                        for jp in range(PT // 2):
                            nc.tensor.matmul(
                                wt[:, n * 512:(n + 1) * 512],
                                lhsT=vpT[:, 2 * jp:2 * jp + 2, m * 128:(m + 1) * 128],
                                rhs=expT[:, 2 * jp:2 * jp + 2, n * 512:(n + 1) * 512],
                                start=(jp == 0), stop=(jp == PT // 2 - 1), perf_mode=DR)
                    nc.scalar.activation(resT[:, m, :], wt,
                                         mybir.ActivationFunctionType.Copy)
                for m in range(CT):
                    tmp = ftpool.tile([128, HW], F32, tag="ft")
                    nc.vector.tensor_mul(tmp, resT[:, m, :], recip)
                    nc.vector.scalar_tensor_tensor(
                        out=xt[:, m, :], in0=tmp, scalar=outb_sb[:, m:m + 1],
                        in1=xt[:, m, :], op0=ADD, op1=ADD)
                    nc.sync.dma_start(
                        out=out_d[bb, m * 128:(m + 1) * 128, :],
                        in_=xt[:, m, :])

            # ---- software pipeline over batches ----
            # Issue order matters for the per-engine FIFOs: norm(bb+1) comes
            # after part2a(bb) so the softmax Exps aren't stuck behind the
            # stats Ln in the ACT queue, and before part2b(bb) so xn(bb+1) is
            # ready the moment the PE finishes av(bb).
            stv_cur = norm_stats(0)
            xn_cur = norm_apply(0, stv_cur)
            for bb in range(B_PER_CORE):
                if bb + 1 < B_PER_CORE:
                    load_x(bb + 1)
                t1, vpT = part1(bb, xn_cur)
                expT, recip = part2a(bb, xn_cur, t1)
                if bb + 1 < B_PER_CORE:
                    stv_next = norm_stats(bb + 1)
                    xn_next = norm_apply(bb + 1, stv_next)
                else:
                    xn_next = None
                part2b(bb, vpT, expT, recip)
                xn_cur = xn_next
    return nc


_NC_CACHE = None


def _q8(v: np.ndarray, scale: float = 1.0) -> np.ndarray:
    """Quantize to TRN e4m3 (saturating at +-240) after scaling."""
    return np.clip(np.asarray(v, np.float64) * scale, -240.0, 240.0).astype(NP8)


def kernel(x, norm_gamma, norm_beta, qkv_w, qkv_b, out_w, out_b):
    global _NC_CACHE
    if _NC_CACHE is None:
        _NC_CACHE = build_nc()
    nc = _NC_CACHE

    x = np.ascontiguousarray(np.asarray(x, np.float32).reshape(B_TOTAL, C, HW))
    qkv_w = np.asarray(qkv_w, np.float64)
    out_w = np.asarray(out_w, np.float64)
    qkv_b = np.asarray(qkv_b, np.float64)
    wq, wk, wv = qkv_w[:C], qkv_w[C:2 * C], qkv_w[2 * C:]
    bq, bk, bv = qkv_b[:C], qkv_b[C:2 * C], qkv_b[2 * C:]

    g8 = np.ascontiguousarray(_q8(wq.T @ wk, WSCALE))           # [c_in, c_out]
    h8 = np.ascontiguousarray(_q8((out_w @ wv).T, WSCALE))      # [c_in, c_out]
    t1b = np.ascontiguousarray((wk.T @ bq).astype(np.float32))
    outb = np.ascontiguousarray(
        (np.asarray(out_b, np.float64) + out_w @ bv).astype(np.float32))
    gamma = np.ascontiguousarray(np.asarray(norm_gamma, np.float32))
    beta = np.ascontiguousarray(np.asarray(norm_beta, np.float32))
    cidx = np.arange(C)
    # each group = 64 channels; selector averages the 64 per-channel stats
    sel = np.ascontiguousarray((cidx[:, None] // (C // GROUPS) == np.arange(GROUPS)[None, :])
                               .astype(np.float32) / (C // GROUPS))
    selT = np.ascontiguousarray((np.arange(GROUPS)[:, None] == cidx[None, :] // (C // GROUPS))
                                .astype(np.float32))

    shared = {"g8": g8, "h8": h8, "t1b": t1b, "outb": outb,
              "gamma": gamma, "beta": beta, "sel": sel, "selT": selT}
    in_maps = [{"x": x[c * B_PER_CORE:(c + 1) * B_PER_CORE], **shared}
               for c in range(N_CORES)]

    trace = bool(int(os.environ.get("KERNEL_TRACE", "0")))
    res = run_bass_kernel_spmd(nc, in_maps, list(range(N_CORES)), trace=trace)
    if trace and res.exec_time_ns is not None:
        print(f"HW exec time: {res.exec_time_ns} ns")
        print(f"(mean across cores: {res.mean_exec_time_ns} ns, "
              f"max core: {res.max_exec_time_core_id})")

    out = np.concatenate([res.results[c]["out"] for c in range(N_CORES)], axis=0)
    return out.reshape(B_TOTAL, C, 32, 32).astype(np.float32)


# revision 33
# speedup vs baseline: 1.1756x; 1.1756x over previous
"""Trainium2 Bass kernel: GroupNorm + single-head self-attention block.

Reference computation (per batch b, x: [C=512, HW=1024] after flattening spatial):
    xn   = groupnorm(x, 8 groups over C, eps=1e-5) * gamma + beta
    qkv  = qkv_w @ xn + qkv_b            # [3C, HW]
    s    = q^T k * C^-0.5
    out  = x + out_w @ (v @ softmax(s)) + out_b

Implementation strategy (fp8 DoubleRow everywhere):
  - Host folds G = Wq^T Wk and H = Wo Wv, so per batch the kernel runs
        t1  = G^T xn                  [C, HW]   (q/k projections fused)
        v'T = xn^T H^T                [HW, C]   (v and out projections fused)
        sT  = xn^T t1                 [HW, HW]  (= k^T q, transposed scores)
        e   = exp(sT * C^-0.5 - 2)              (shift cancels in normalization)
        cs  = ones^T e  (colsum), recip = 1/cs
        res = v'^ e                   [C, HW]
        out = x + res * recip + outb_eff
    This removes 1/3 of the matmul MACs and their PSUM->SBUF copies.
  - All big matmuls run as float8e4 (TRN E4M3, +-240) with
    MatmulPerfMode.DoubleRow: K=256 per instruction at the same wall cost as a
    K=128 fp32r matmul (measured 244ns vs 254ns for N=512) -> 2.08x PE rate.
    Weights are pre-scaled x16 on host so their sigma~0.044 lands in e4m3's
    normal range; the 1/16 is folded into the post-matmul activation scale.
    Measured end-to-end rel err ~9e-3 vs the 2e-2 gate (numpy-sim predicted
    8.6e-3).
  - qkv biases are handled exactly: the bq-side rank-1 term folds into t1's
    activation bias (Wk^T bq); the per-pixel-i terms scale whole columns of e
    and cancel in the colsum division; v/out biases fold into outb_eff.

Sharding: data-parallel over batch, 32 batches / 8 cores = 4 per core.
"""

import json
import os

import numpy as np
import ml_dtypes

import concourse.bass as bass
import concourse.mybir as mybir
import concourse.tile as tile
from concourse.bass_utils import run_bass_kernel_spmd


def _spill_multiwaits(raw: bytes) -> bytes:
    """Walrus in this toolchain accepts only one sync-wait command per
    instruction descriptor. Spill extra on_wait entries onto single-wait
    EventSemaphore instructions inserted immediately before, on the same
    engine queue (the exact pattern Tile's own barriers use), which is
    semantically identical: the queue blocks at the same point either way.
    """
    j = json.loads(raw)
    n = 0
    for fn in j.get("functions", []):
        for blk in fn.get("blocks", []):
            out = []
            for inst in blk.get("instructions", []):
                si = inst.get("sync_info") or {}
                waits = si.get("on_wait") or []
                if len(waits) > 1 and inst.get("engine"):
                    for spilled in waits[:-1]:
                        n += 1
                        out.append({
                            "debug": inst.get("debug", 0),
                            "engine": inst["engine"],
                            "ins": [],
                            "name": f"{inst['name']}-sw{n}",
                            "opcode": "EventSemaphore",
                            "outs": [],
                            "sync_info": {"on_update": [], "on_wait": [spilled]},
                        })
                    si["on_wait"] = waits[-1:]
                out.append(inst)
            blk["instructions"] = out
    return json.dumps(j).encode()


_orig_to_json_bytes = bass.Bass.to_json_bytes


def _patched_to_json_bytes(self):
    return _spill_multiwaits(_orig_to_json_bytes(self))


bass.Bass.to_json_bytes = _patched_to_json_bytes

F32 = mybir.dt.float32
F32R = mybir.dt.float32r  # stats-path matmuls only
FP8 = mybir.dt.float8e4
DR = mybir.MatmulPerfMode.DoubleRow
NP8 = ml_dtypes.float8_e4m3  # TRN e4m3: max normal +-240

N_CORES = 8
B_TOTAL = 32
B_PER_CORE = B_TOTAL // N_CORES
C = 512
HW = 1024
GROUPS = 8
EPS = 1e-5
SCALE = float(C) ** -0.5
EXP_SHIFT = -2.0     # cancels in colsum division; keeps e < 34 << 240
WSCALE = 16.0        # weight pre-scale into e4m3 normal range

CT = C // 128   # 4 channel tiles
PT = HW // 128  # 8 pixel tiles
NB = HW // 512  # 2 free-dim blocks of 512
MUL = mybir.AluOpType.mult
ADD = mybir.AluOpType.add


def build_nc():
    nc = bass.Bass()

    x_d = nc.dram_tensor("x", [B_PER_CORE, C, HW], F32, kind="ExternalInput")
    g_d = nc.dram_tensor("g8", [C, C], FP8, kind="ExternalInput")     # 16*Wq^T Wk  [c_in, c_out]
    h_d = nc.dram_tensor("h8", [C, C], FP8, kind="ExternalInput")     # 16*(Wo Wv)^T [c_in, c_out]
    t1b_d = nc.dram_tensor("t1b", [C], F32, kind="ExternalInput")     # Wk^T bq
    outb_d = nc.dram_tensor("outb", [C], F32, kind="ExternalInput")   # out_b + Wo bv
    gamma_d = nc.dram_tensor("gamma", [C], F32, kind="ExternalInput")
    beta_d = nc.dram_tensor("beta", [C], F32, kind="ExternalInput")
    sel_d = nc.dram_tensor("sel", [C, GROUPS], F32, kind="ExternalInput")
    selT_d = nc.dram_tensor("selT", [GROUPS, C], F32, kind="ExternalInput")
    out_d = nc.dram_tensor("out", [B_PER_CORE, C, HW], F32, kind="ExternalOutput")
    warmdump_d = nc.dram_tensor("warmdump", [128, 4], F32)

    with tile.TileContext(nc) as tc:
        with (
            tc.tile_pool(name="wpool", bufs=1) as wpool,
            tc.tile_pool(name="xpool", bufs=3) as xpool,
            tc.tile_pool(name="xnpool", bufs=2) as xnpool,
            tc.tile_pool(name="t1pool", bufs=1) as t1pool,
            tc.tile_pool(name="vppool", bufs=1) as vppool,
            tc.tile_pool(name="expool", bufs=1) as expool,
            tc.tile_pool(name="rpool", bufs=2) as rpool,
            tc.tile_pool(name="respool", bufs=1) as respool,
            tc.tile_pool(name="ftpool", bufs=2) as ftpool,
            tc.tile_pool(name="spool", bufs=2) as spool,
            tc.tile_pool(name="wps", bufs=3, space=bass.MemorySpace.PSUM) as wps,
            tc.tile_pool(name="stps", bufs=1, space=bass.MemorySpace.PSUM) as stps,
        ):
            xts = {}

            def load_x(bb):
                xt = xpool.tile([128, CT, HW], F32, tag="xt")
                xts[bb] = xt
                # per-c-tile chunks so bn_stats can start before the full
                # load; issued from the otherwise-idle gpsimd queue so they
                # don't serialize behind the output stores on the sync queue
                for t in range(CT):
                    nc.gpsimd.dma_start(
                        out=xt[:, t],
                        in_=x_d[bb, t * 128:(t + 1) * 128, :])
                return xt

            # x(0) first: its consumer chain (stats -> xn -> t1) is the
            # critical path to the first big matmul
            load_x(0)

            # ---- tiny constants (cheap DMAs / memsets) ----
            eps_sb = wpool.tile([128, 1], F32)
            nc.vector.memset(eps_sb, EPS)
            shift_sb = wpool.tile([128, 1], F32)
            nc.vector.memset(shift_sb, EXP_SHIFT)
            ones_st = wpool.tile([128, 256], F32)
            nc.vector.memset(ones_st, 1.0)
            ones8 = wpool.tile([128, 2, 128], FP8)
            nc.vector.tensor_copy(ones8, ones_st.rearrange("p (a b) -> p a b", a=2))
            # HAM warmup: keep the PE busy under the startup DMA window so the
            # clock gate reaches 8/8 (2.4GHz) before the first real matmul.
            warm_st = wpool.tile([128, 512], F32)
            nc.vector.memset(warm_st, 0.0)
            warm_rhs = wpool.tile([128, 512], F32R)
            nc.vector.tensor_copy(warm_rhs, warm_st)
            ones_r = wpool.tile([128, 128], F32R)
            nc.vector.tensor_copy(ones_r, ones_st[:, 0:128])
            warm_ps = stps.tile([128, 512], F32, tag="gps")
            for w in range(35):
                nc.tensor.matmul(warm_ps, lhsT=ones_r, rhs=warm_rhs,
                                 start=True, stop=True)
            warm_out = wpool.tile([128, 4], F32)
            nc.vector.tensor_copy(warm_out, warm_ps[:, 0:4])
            nc.sync.dma_start(out=warmdump_d[:, :], in_=warm_out)

            sel_st = wpool.tile([128, CT, GROUPS], F32)
            nc.sync.dma_start(out=sel_st, in_=sel_d.rearrange("(t p) g -> p t g", p=128))
            sel_sb = wpool.tile([128, CT, GROUPS], F32R)
            nc.vector.tensor_copy(sel_sb, sel_st)
            selT_st = wpool.tile([GROUPS, C], F32)
            nc.sync.dma_start(out=selT_st, in_=selT_d[:, :])
            selT_sb = wpool.tile([GROUPS, C], F32R)
            nc.vector.tensor_copy(selT_sb, selT_st)
            t1b_sb = wpool.tile([128, CT], F32)
            nc.sync.dma_start(out=t1b_sb, in_=t1b_d.rearrange("(m p) -> p m", p=128))
            outb_sb = wpool.tile([128, CT], F32)
            nc.sync.dma_start(out=outb_sb, in_=outb_d.rearrange("(m p) -> p m", p=128))
            gamma_sb = wpool.tile([128, CT], F32)
            nc.sync.dma_start(out=gamma_sb, in_=gamma_d.rearrange("(m p) -> p m", p=128))
            beta_sb = wpool.tile([128, CT], F32)
            nc.sync.dma_start(out=beta_sb, in_=beta_d.rearrange("(m p) -> p m", p=128))

            # ---- fused weights (fp8, pre-scaled x16 on host) ----
            g_sb = wpool.tile([128, CT, C], FP8)
            h_sb = wpool.tile([128, CT, C], FP8)
            g_r = g_d.rearrange("(t p) o -> p t o", p=128)
            h_r = h_d.rearrange("(t p) o -> p t o", p=128)
            for t in range(CT):
                nc.sync.dma_start(out=g_sb[:, t], in_=g_r[:, t])
                nc.sync.dma_start(out=h_sb[:, t], in_=h_r[:, t])

            def norm_stats(bb):
                """GroupNorm stats for batch bb -> per-channel (scale, shift)."""
                xt = xts[bb]
                stats3 = spool.tile([128, CT, 4], F32, tag="stats3")
                nc.vector.memset(stats3, 0.0)
                for t in range(CT):
                    st6 = spool.tile([128, 2, 6], F32, tag="st6")
                    for sg in range(2):
                        nc.vector.bn_stats(out=st6[:, sg], in_=xt[:, t, sg * 512:(sg + 1) * 512])
                    nc.vector.bn_aggr(out=stats3[:, t, 0:2], in_=st6)
                    nc.vector.tensor_mul(stats3[:, t, 2:3], stats3[:, t, 0:1], stats3[:, t, 0:1])
                stats3r = spool.tile([128, CT, 4], F32R, tag="stats3r")
                nc.vector.tensor_copy(stats3r, stats3)
                gps = stps.tile([GROUPS, 4], F32, tag="gps")
                for t in range(CT):
                    nc.tensor.matmul(gps, lhsT=sel_sb[:, t], rhs=stats3r[:, t],
                                     start=(t == 0), stop=(t == CT - 1))
                # group var = E[var_c] + E[mean_c^2] - E[mean_c]^2 ; then rstd
                gsb = spool.tile([GROUPS, 4], F32, tag="gsb")
                nc.vector.tensor_copy(gsb, gps)
                gs = spool.tile([GROUPS, 4], F32, tag="gs")
                nc.vector.memset(gs, 0.0)
                tmp8 = spool.tile([GROUPS, 1], F32, tag="tmp8")
                nc.vector.tensor_mul(tmp8, gsb[:, 0:1], gsb[:, 0:1])
                nc.vector.tensor_add(gs[:, 1:2], gsb[:, 1:2], gsb[:, 2:3])
                nc.vector.tensor_sub(gs[:, 1:2], gs[:, 1:2], tmp8)
                # rstd = exp(-0.5*ln(var+eps)): Ln/Exp share an ACT table set
                # with the softmax Exp, avoiding 2 x 1.3us table reloads/batch
                # that Sqrt (different set) would trigger.
                nc.scalar.activation(gs[:, 1:2], gs[:, 1:2],
                                     mybir.ActivationFunctionType.Ln,
                                     bias=eps_sb[:GROUPS])
                nc.scalar.activation(gs[:, 1:2], gs[:, 1:2],
                                     mybir.ActivationFunctionType.Exp,
                                     scale=-0.5)
                nc.vector.tensor_copy(gs[:, 0:1], gsb[:, 0:1])
                # broadcast group stats back to channel partitions
                gsr = spool.tile([GROUPS, 4], F32R, tag="gsr")
                nc.vector.tensor_copy(gsr, gs)
                csps = stps.tile([128, CT, 4], F32, tag="csps")
                for t in range(CT):
                    nc.tensor.matmul(csps[:, t], lhsT=selT_sb[:, t * 128:(t + 1) * 128],
                                     rhs=gsr, start=True, stop=True)
                # per-channel affine: xn = x * s + tt
                stv = spool.tile([128, CT, 2], F32, tag="stv")
                for t in range(CT):
                    tmpc = spool.tile([128, 1], F32, tag="tmpc")
                    nc.vector.tensor_mul(stv[:, t, 0:1], csps[:, t, 1:2], gamma_sb[:, t:t + 1])
                    nc.vector.tensor_mul(tmpc, csps[:, t, 0:1], stv[:, t, 0:1])
                    nc.vector.tensor_sub(stv[:, t, 1:2], beta_sb[:, t:t + 1], tmpc)
                return stv

            def norm_apply(bb, stv):
                """xn = x*s + t, quantized straight to fp8 for the matmuls."""
                xt = xts[bb]
                xn = xnpool.tile([128, CT, HW], FP8, tag="xn")
                # all first-halves first: the first t1 accumulation group only
                # reads columns 0:512 of c-tiles 0..1, so it can start early
                for h in range(NB):
                    for t in range(CT):
                        nc.vector.tensor_scalar(
                            out=xn[:, t, h * 512:(h + 1) * 512],
                            in0=xt[:, t, h * 512:(h + 1) * 512],
                            scalar1=stv[:, t, 0:1], scalar2=stv[:, t, 1:2],
                            op0=MUL, op1=ADD)
                return xn

            def part1(bb, xn):
                """t1 = G^T xn and v'T = xn^T H^T, both fp8."""
                t1 = t1pool.tile([128, CT, HW], FP8, tag="t1")
                for m in range(CT):
                    wt = wps.tile([128, 2 * 512], F32, tag="mm")
                    for n in range(NB):
                        for tp in range(2):
                            nc.tensor.matmul(
                                wt[:, n * 512:(n + 1) * 512],
                                lhsT=g_sb[:, 2 * tp:2 * tp + 2, m * 128:(m + 1) * 128],
                                rhs=xn[:, 2 * tp:2 * tp + 2, n * 512:(n + 1) * 512],
                                start=(tp == 0), stop=(tp == 1), perf_mode=DR)
                    nc.scalar.activation(t1[:, m, :], wt,
                                         mybir.ActivationFunctionType.Identity,
                                         bias=t1b_sb[:, m:m + 1], scale=1.0 / WSCALE)
                vpT = vppool.tile([128, PT, C], FP8, tag="vpT")
                for pp in range(PT // 2):
                    wt = wps.tile([128, 2 * 512], F32, tag="mm")
                    for i in range(2):
                        p = 2 * pp + i
                        for tp in range(2):
                            nc.tensor.matmul(
                                wt[:, i * 512:(i + 1) * 512],
                                lhsT=xn[:, 2 * tp:2 * tp + 2, p * 128:(p + 1) * 128],
                                rhs=h_sb[:, 2 * tp:2 * tp + 2, :],
                                start=(tp == 0), stop=(tp == 1), perf_mode=DR)
                    nc.scalar.activation(vpT[:, 2 * pp:2 * pp + 2, :],
                                         wt.rearrange("p (a b) -> p a b", a=2),
                                         mybir.ActivationFunctionType.Copy,
                                         scale=1.0 / WSCALE)
                return t1, vpT

            def part2a(bb, xn, t1):
                """transposed scores -> exp (fp8) -> colsum -> recip.

                """
                expT = expool.tile([128, PT, HW], FP8, tag="expT")
                for jm in range(PT):
                    wt = wps.tile([128, 2 * 512], F32, tag="mm")
                    for n in range(NB):
                        for tp in range(2):
                            nc.tensor.matmul(
                                wt[:, n * 512:(n + 1) * 512],
                                lhsT=xn[:, 2 * tp:2 * tp + 2, jm * 128:(jm + 1) * 128],
                                rhs=t1[:, 2 * tp:2 * tp + 2, n * 512:(n + 1) * 512],
                                start=(tp == 0), stop=(tp == 1), perf_mode=DR)
                    nc.scalar.activation(expT[:, jm, :], wt,
                                         mybir.ActivationFunctionType.Exp,
                                         bias=shift_sb, scale=SCALE)
                colp = wps.tile([128, 2 * 512], F32, tag="mm")
                for n in range(NB):
                    for jp in range(PT // 2):
                        nc.tensor.matmul(
                            colp[:, n * 512:(n + 1) * 512],
                            lhsT=ones8,
                            rhs=expT[:, 2 * jp:2 * jp + 2, n * 512:(n + 1) * 512],
                            start=(jp == 0), stop=(jp == PT // 2 - 1), perf_mode=DR)
                # recip = exp(-ln(colsum)) on ACT: ln and exp share the
                # resident table set, and the exact DVE reciprocal (6.5us for
                # [128,1024]) would gate the av-stage PSUM rotation.
                lncs = rpool.tile([128, HW], F32, tag="lncs")
                nc.scalar.activation(lncs, colp,
                                     mybir.ActivationFunctionType.Ln)
                recip = rpool.tile([128, HW], F32, tag="recip")
                nc.scalar.activation(recip, lncs,
                                     mybir.ActivationFunctionType.Exp,
                                     scale=-1.0)
                return expT, recip

            def part2b(bb, vpT, expT, recip):
                """res = v' e, normalize, bias, residual, store."""
                xt = xts.pop(bb)
                # Drain every av psum to SBUF via a cheap ACT copy right away:
                # the psum slot frees in ~1.1us regardless of where the DVE is
                # in its queue, so the next batch's t1/scores rotation never
                # starves on a psum bank held hostage by a late normalize.
                resT = respool.tile([128, CT, HW], F32, tag="resT")
                for m in range(CT):
                    wt = wps.tile([128, 2 * 512], F32, tag="mm")
                    for n in range(NB):
                        for jp in range(PT // 2):
                            nc.tensor.matmul(
                                wt[:, n * 512:(n + 1) * 512],
                                lhsT=vpT[:, 2 * jp:2 * jp + 2, m * 128:(m + 1) * 128],
                                rhs=expT[:, 2 * jp:2 * jp + 2, n * 512:(n + 1) * 512],
                                start=(jp == 0), stop=(jp == PT // 2 - 1), perf_mode=DR)
                    nc.scalar.activation(resT[:, m, :], wt,
                                         mybir.ActivationFunctionType.Copy)
                for m in range(CT):
                    tmp = ftpool.tile([128, HW], F32, tag="ft")
                    nc.vector.tensor_mul(tmp, resT[:, m, :], recip)
                    nc.vector.scalar_tensor_tensor(
                        out=xt[:, m, :], in0=tmp, scalar=outb_sb[:, m:m + 1],
                        in1=xt[:, m, :], op0=ADD, op1=ADD)
                    nc.sync.dma_start(
                        out=out_d[bb, m * 128:(m + 1) * 128, :],
                        in_=xt[:, m, :])

            # ---- software pipeline over batches ----
            # Issue order matters for the per-engine FIFOs: norm(bb+1) comes
            # after part2a(bb) so the softmax Exps aren't stuck behind the
            # stats Ln in the ACT queue, and before part2b(bb) so xn(bb+1) is
            # ready the moment the PE finishes av(bb).
            stv_cur = norm_stats(0)
            xn_cur = norm_apply(0, stv_cur)
            for bb in range(B_PER_CORE):
                if bb + 1 < B_PER_CORE:
                    load_x(bb + 1)
                t1, vpT = part1(bb, xn_cur)
                expT, recip = part2a(bb, xn_cur, t1)
                if bb + 1 < B_PER_CORE:
                    stv_next = norm_stats(bb + 1)
                    xn_next = norm_apply(bb + 1, stv_next)
                else:
                    xn_next = None
                part2b(bb, vpT, expT, recip)
                xn_cur = xn_next
    return nc


_NC_CACHE = None


def _q8(v: np.ndarray, scale: float = 1.0) -> np.ndarray:
    """Quantize to TRN e4m3 (saturating at +-240) after scaling."""
    return np.clip(np.asarray(v, np.float64) * scale, -240.0, 240.0).astype(NP8)


def kernel(x, norm_gamma, norm_beta, qkv_w, qkv_b, out_w, out_b):
    global _NC_CACHE
    if _NC_CACHE is None:
        _NC_CACHE = build_nc()
    nc = _NC_CACHE

    x = np.ascontiguousarray(np.asarray(x, np.float32).reshape(B_TOTAL, C, HW))
    qkv_w = np.asarray(qkv_w, np.float64)
    out_w = np.asarray(out_w, np.float64)
    qkv_b = np.asarray(qkv_b, np.float64)
    wq, wk, wv = qkv_w[:C], qkv_w[C:2 * C], qkv_w[2 * C:]
    bq, bk, bv = qkv_b[:C], qkv_b[C:2 * C], qkv_b[2 * C:]

    g8 = np.ascontiguousarray(_q8(wq.T @ wk, WSCALE))           # [c_in, c_out]
    h8 = np.ascontiguousarray(_q8((out_w @ wv).T, WSCALE))      # [c_in, c_out]
    t1b = np.ascontiguousarray((wk.T @ bq).astype(np.float32))
    outb = np.ascontiguousarray(
        (np.asarray(out_b, np.float64) + out_w @ bv).astype(np.float32))
    gamma = np.ascontiguousarray(np.asarray(norm_gamma, np.float32))
    beta = np.ascontiguousarray(np.asarray(norm_beta, np.float32))
    cidx = np.arange(C)
    # each group = 64 channels; selector averages the 64 per-channel stats
    sel = np.ascontiguousarray((cidx[:, None] // (C // GROUPS) == np.arange(GROUPS)[None, :])
                               .astype(np.float32) / (C // GROUPS))
    selT = np.ascontiguousarray((np.arange(GROUPS)[:, None] == cidx[None, :] // (C // GROUPS))
                                .astype(np.float32))

    shared = {"g8": g8, "h8": h8, "t1b": t1b, "outb": outb,
              "gamma": gamma, "beta": beta, "sel": sel, "selT": selT}
    in_maps = [{"x": x[c * B_PER_CORE:(c + 1) * B_PER_CORE], **shared}
               for c in range(N_CORES)]

    trace = bool(int(os.environ.get("KERNEL_TRACE", "0")))
    res = run_bass_kernel_spmd(nc, in_maps, list(range(N_CORES)), trace=trace)
    if trace and res.exec_time_ns is not None:
        print(f"HW exec time: {res.exec_time_ns} ns")
        print(f"(mean across cores: {res.mean_exec_time_ns} ns, "
              f"max core: {res.max_exec_time_core_id})")

    out = np.concatenate([res.results[c]["out"] for c in range(N_CORES)], axis=0)
    return out.reshape(B_TOTAL, C, 32, 32).astype(np.float32)


# revision 35
# speedup vs baseline: 1.1925x; 1.0144x over previous
"""Trainium2 Bass kernel: GroupNorm + single-head self-attention block.

Reference computation (per batch b, x: [C=512, HW=1024] after flattening spatial):
    xn   = groupnorm(x, 8 groups over C, eps=1e-5) * gamma + beta
    qkv  = qkv_w @ xn + qkv_b            # [3C, HW]
    s    = q^T k * C^-0.5
    out  = x + out_w @ (v @ softmax(s)) + out_b

Implementation strategy (fp8 DoubleRow everywhere):
  - Host folds G = Wq^T Wk and H = Wo Wv, so per batch the kernel runs
        t1  = G^T xn                  [C, HW]   (q/k projections fused)
        v'T = xn^T H^T                [HW, C]   (v and out projections fused)
        sT  = xn^T t1                 [HW, HW]  (= k^T q, transposed scores)
        e   = exp(sT * C^-0.5 - 2)              (shift cancels in normalization)
        cs  = ones^T e  (colsum), recip = 1/cs
        res = v'^ e                   [C, HW]
        out = x + res * recip + outb_eff
    This removes 1/3 of the matmul MACs and their PSUM->SBUF copies.
  - All big matmuls run as float8e4 (TRN E4M3, +-240) with
    MatmulPerfMode.DoubleRow: K=256 per instruction at the same wall cost as a
    K=128 fp32r matmul (measured 244ns vs 254ns for N=512) -> 2.08x PE rate.
    Weights are pre-scaled x16 on host so their sigma~0.044 lands in e4m3's
    normal range; the 1/16 is folded into the post-matmul activation scale.
    Measured end-to-end rel err ~9e-3 vs the 2e-2 gate (numpy-sim predicted
    8.6e-3).
  - qkv biases are handled exactly: the bq-side rank-1 term folds into t1's
    activation bias (Wk^T bq); the per-pixel-i terms scale whole columns of e
    and cancel in the colsum division; v/out biases fold into outb_eff.

Sharding: data-parallel over batch, 32 batches / 8 cores = 4 per core.
"""

import json
import os

import numpy as np
import ml_dtypes

import concourse.bass as bass
import concourse.mybir as mybir
import concourse.tile as tile
from concourse.bass_utils import run_bass_kernel_spmd


def _spill_multiwaits(raw: bytes) -> bytes:
    """Walrus in this toolchain accepts only one sync-wait command per
    instruction descriptor. Spill extra on_wait entries onto single-wait
    EventSemaphore instructions inserted immediately before, on the same
    engine queue (the exact pattern Tile's own barriers use), which is
    semantically identical: the queue blocks at the same point either way.
    """
    j = json.loads(raw)
    n = 0
    for fn in j.get("functions", []):
        for blk in fn.get("blocks", []):
            out = []
            for inst in blk.get("instructions", []):
                si = inst.get("sync_info") or {}
                waits = si.get("on_wait") or []
                if len(waits) > 1 and inst.get("engine"):
                    for spilled in waits[:-1]:
                        n += 1
                        out.append({
                            "debug": inst.get("debug", 0),
                            "engine": inst["engine"],
                            "ins": [],
                            "name": f"{inst['name']}-sw{n}",
                            "opcode": "EventSemaphore",
                            "outs": [],
                            "sync_info": {"on_update": [], "on_wait": [spilled]},
                        })
                    si["on_wait"] = waits[-1:]
                out.append(inst)
            blk["instructions"] = out
    return json.dumps(j).encode()


_orig_to_json_bytes = bass.Bass.to_json_bytes


def _patched_to_json_bytes(self):
    return _spill_multiwaits(_orig_to_json_bytes(self))


bass.Bass.to_json_bytes = _patched_to_json_bytes

F32 = mybir.dt.float32
F32R = mybir.dt.float32r  # stats-path matmuls only
FP8 = mybir.dt.float8e4
DR = mybir.MatmulPerfMode.DoubleRow
NP8 = ml_dtypes.float8_e4m3  # TRN e4m3: max normal +-240

N_CORES = 8
B_TOTAL = 32
B_PER_CORE = B_TOTAL // N_CORES
C = 512
HW = 1024
GROUPS = 8
EPS = 1e-5
SCALE = float(C) ** -0.5
EXP_SHIFT = -2.0     # cancels in colsum division; keeps e < 34 << 240
WSCALE = 16.0        # weight pre-scale into e4m3 normal range

CT = C // 128   # 4 channel tiles
PT = HW // 128  # 8 pixel tiles
NB = HW // 512  # 2 free-dim blocks of 512
MUL = mybir.AluOpType.mult
ADD = mybir.AluOpType.add


def build_nc():
    nc = bass.Bass()

    x_d = nc.dram_tensor("x", [B_PER_CORE, C, HW], F32, kind="ExternalInput")
    g_d = nc.dram_tensor("g8", [C, C], FP8, kind="ExternalInput")     # 16*Wq^T Wk  [c_in, c_out]
    h_d = nc.dram_tensor("h8", [C, C], FP8, kind="ExternalInput")     # 16*(Wo Wv)^T [c_in, c_out]
    t1b_d = nc.dram_tensor("t1b", [C], F32, kind="ExternalInput")     # Wk^T bq
    outb_d = nc.dram_tensor("outb", [C], F32, kind="ExternalInput")   # out_b + Wo bv
    gamma_d = nc.dram_tensor("gamma", [C], F32, kind="ExternalInput")
    beta_d = nc.dram_tensor("beta", [C], F32, kind="ExternalInput")
    sel_d = nc.dram_tensor("sel", [C, GROUPS], F32, kind="ExternalInput")
    selT_d = nc.dram_tensor("selT", [GROUPS, C], F32, kind="ExternalInput")
    out_d = nc.dram_tensor("out", [B_PER_CORE, C, HW], F32, kind="ExternalOutput")
    warmdump_d = nc.dram_tensor("warmdump", [128, 4], F32)

    with tile.TileContext(nc) as tc:
        with (
            tc.tile_pool(name="wpool", bufs=1) as wpool,
            tc.tile_pool(name="xpool", bufs=3) as xpool,
            tc.tile_pool(name="xnpool", bufs=2) as xnpool,
            tc.tile_pool(name="t1pool", bufs=1) as t1pool,
            tc.tile_pool(name="vppool", bufs=1) as vppool,
            tc.tile_pool(name="expool", bufs=1) as expool,
            tc.tile_pool(name="rpool", bufs=2) as rpool,
            tc.tile_pool(name="respool", bufs=1) as respool,
            tc.tile_pool(name="ftpool", bufs=2) as ftpool,
            tc.tile_pool(name="spool", bufs=2) as spool,
            tc.tile_pool(name="wps", bufs=3, space=bass.MemorySpace.PSUM) as wps,
            tc.tile_pool(name="stps", bufs=1, space=bass.MemorySpace.PSUM) as stps,
        ):
            xts = {}

            def load_x(bb):
                xt = xpool.tile([128, CT, HW], F32, tag="xt")
                xts[bb] = xt
                # per-c-tile chunks so bn_stats can start before the full
                # load; issued from the otherwise-idle gpsimd queue so they
                # don't serialize behind the output stores on the sync queue
                for t in range(CT):
                    nc.gpsimd.dma_start(
                        out=xt[:, t],
                        in_=x_d[bb, t * 128:(t + 1) * 128, :])
                return xt

            # x(0) first: its consumer chain (stats -> xn -> t1) is the
            # critical path to the first big matmul
            load_x(0)

            # ---- tiny constants (cheap DMAs / memsets) ----
            eps_sb = wpool.tile([128, 1], F32)
            nc.vector.memset(eps_sb, EPS)
            shift_sb = wpool.tile([128, 1], F32)
            nc.vector.memset(shift_sb, EXP_SHIFT)
            ones_st = wpool.tile([128, 256], F32)
            nc.vector.memset(ones_st, 1.0)
            ones8 = wpool.tile([128, 2, 128], FP8)
            nc.vector.tensor_copy(ones8, ones_st.rearrange("p (a b) -> p a b", a=2))
            # HAM warmup: keep the PE busy under the startup DMA window so the
            # clock gate reaches 8/8 (2.4GHz) before the first real matmul.
            warm_st = wpool.tile([128, 512], F32)
            nc.vector.memset(warm_st, 0.0)
            warm_rhs = wpool.tile([128, 512], F32R)
            nc.vector.tensor_copy(warm_rhs, warm_st)
            ones_r = wpool.tile([128, 128], F32R)
            nc.vector.tensor_copy(ones_r, ones_st[:, 0:128])
            warm_ps = stps.tile([128, 512], F32, tag="gps")
            for w in range(35):
                nc.tensor.matmul(warm_ps, lhsT=ones_r, rhs=warm_rhs,
                                 start=True, stop=True)
            warm_out = wpool.tile([128, 4], F32)
            nc.vector.tensor_copy(warm_out, warm_ps[:, 0:4])
            nc.sync.dma_start(out=warmdump_d[:, :], in_=warm_out)

            sel_st = wpool.tile([128, CT, GROUPS], F32)
            nc.sync.dma_start(out=sel_st, in_=sel_d.rearrange("(t p) g -> p t g", p=128))
            sel_sb = wpool.tile([128, CT, GROUPS], F32R)
            nc.vector.tensor_copy(sel_sb, sel_st)
            selT_st = wpool.tile([GROUPS, C], F32)
            nc.sync.dma_start(out=selT_st, in_=selT_d[:, :])
            selT_sb = wpool.tile([GROUPS, C], F32R)
            nc.vector.tensor_copy(selT_sb, selT_st)
            t1b_sb = wpool.tile([128, CT], F32)
            nc.sync.dma_start(out=t1b_sb, in_=t1b_d.rearrange("(m p) -> p m", p=128))
            outb_sb = wpool.tile([128, CT], F32)
            nc.sync.dma_start(out=outb_sb, in_=outb_d.rearrange("(m p) -> p m", p=128))
            gamma_sb = wpool.tile([128, CT], F32)
            nc.sync.dma_start(out=gamma_sb, in_=gamma_d.rearrange("(m p) -> p m", p=128))
            beta_sb = wpool.tile([128, CT], F32)
            nc.sync.dma_start(out=beta_sb, in_=beta_d.rearrange("(m p) -> p m", p=128))

            # ---- fused weights (fp8, pre-scaled x16 on host) ----
            g_sb = wpool.tile([128, CT, C], FP8)
            h_sb = wpool.tile([128, CT, C], FP8)
            g_r = g_d.rearrange("(t p) o -> p t o", p=128)
            h_r = h_d.rearrange("(t p) o -> p t o", p=128)
            for t in range(CT):
                nc.sync.dma_start(out=g_sb[:, t], in_=g_r[:, t])
                nc.sync.dma_start(out=h_sb[:, t], in_=h_r[:, t])

            def norm_bn(bb):
                """Pure-DVE part of the groupnorm stats (bn over x). Issued a
                full iteration ahead of use so it never sits behind the
                normalize work in the DVE queue."""
                xt = xts[bb]
                stats3 = spool.tile([128, CT, 4], F32, tag="stats3")
                nc.vector.memset(stats3, 0.0)
                for t in range(CT):
                    st6 = spool.tile([128, 2, 6], F32, tag="st6")
                    for sg in range(2):
                        nc.vector.bn_stats(out=st6[:, sg], in_=xt[:, t, sg * 512:(sg + 1) * 512])
                    nc.vector.bn_aggr(out=stats3[:, t, 0:2], in_=st6)
                    nc.vector.tensor_mul(stats3[:, t, 2:3], stats3[:, t, 0:1], stats3[:, t, 0:1])
                stats3r = spool.tile([128, CT, 4], F32R, tag="stats3r")
                nc.vector.tensor_copy(stats3r, stats3)
                return stats3r

            def norm_stats(bb, stats3r):
                """Group aggregation + rstd + per-channel (scale, shift)."""
                gps = stps.tile([GROUPS, 4], F32, tag="gps")
                for t in range(CT):
                    nc.tensor.matmul(gps, lhsT=sel_sb[:, t], rhs=stats3r[:, t],
                                     start=(t == 0), stop=(t == CT - 1))
                # group var = E[var_c] + E[mean_c^2] - E[mean_c]^2 ; then rstd
                gsb = spool.tile([GROUPS, 4], F32, tag="gsb")
                nc.vector.tensor_copy(gsb, gps)
                gs = spool.tile([GROUPS, 4], F32, tag="gs")
                nc.vector.memset(gs, 0.0)
                tmp8 = spool.tile([GROUPS, 1], F32, tag="tmp8")
                nc.vector.tensor_mul(tmp8, gsb[:, 0:1], gsb[:, 0:1])
                nc.vector.tensor_add(gs[:, 1:2], gsb[:, 1:2], gsb[:, 2:3])
                nc.vector.tensor_sub(gs[:, 1:2], gs[:, 1:2], tmp8)
                # rstd = exp(-0.5*ln(var+eps)): Ln/Exp share an ACT table set
                # with the softmax Exp, avoiding 2 x 1.3us table reloads/batch
                # that Sqrt (different set) would trigger.
                nc.scalar.activation(gs[:, 1:2], gs[:, 1:2],
                                     mybir.ActivationFunctionType.Ln,
                                     bias=eps_sb[:GROUPS])
                nc.scalar.activation(gs[:, 1:2], gs[:, 1:2],
                                     mybir.ActivationFunctionType.Exp,
                                     scale=-0.5)
                nc.vector.tensor_copy(gs[:, 0:1], gsb[:, 0:1])
                # broadcast group stats back to channel partitions
                gsr = spool.tile([GROUPS, 4], F32R, tag="gsr")
                nc.vector.tensor_copy(gsr, gs)
                csps = stps.tile([128, CT, 4], F32, tag="csps")
                for t in range(CT):
                    nc.tensor.matmul(csps[:, t], lhsT=selT_sb[:, t * 128:(t + 1) * 128],
                                     rhs=gsr, start=True, stop=True)
                # per-channel affine: xn = x * s + tt
                stv = spool.tile([128, CT, 2], F32, tag="stv")
                for t in range(CT):
                    tmpc = spool.tile([128, 1], F32, tag="tmpc")
                    nc.vector.tensor_mul(stv[:, t, 0:1], csps[:, t, 1:2], gamma_sb[:, t:t + 1])
                    nc.vector.tensor_mul(tmpc, csps[:, t, 0:1], stv[:, t, 0:1])
                    nc.vector.tensor_sub(stv[:, t, 1:2], beta_sb[:, t:t + 1], tmpc)
                return stv

            def norm_apply(bb, stv):
                """xn = x*s + t, quantized straight to fp8 for the matmuls."""
                xt = xts[bb]
                xn = xnpool.tile([128, CT, HW], FP8, tag="xn")
                # all first-halves first: the first t1 accumulation group only
                # reads columns 0:512 of c-tiles 0..1, so it can start early
                for h in range(NB):
                    for t in range(CT):
                        nc.vector.tensor_scalar(
                            out=xn[:, t, h * 512:(h + 1) * 512],
                            in0=xt[:, t, h * 512:(h + 1) * 512],
                            scalar1=stv[:, t, 0:1], scalar2=stv[:, t, 1:2],
                            op0=MUL, op1=ADD)
                return xn

            def part1(bb, xn):
                """t1 = G^T xn and v'T = xn^T H^T, both fp8."""
                t1 = t1pool.tile([128, CT, HW], FP8, tag="t1")
                for m in range(CT):
                    wt = wps.tile([128, 2 * 512], F32, tag="mm")
                    for n in range(NB):
                        for tp in range(2):
                            nc.tensor.matmul(
                                wt[:, n * 512:(n + 1) * 512],
                                lhsT=g_sb[:, 2 * tp:2 * tp + 2, m * 128:(m + 1) * 128],
                                rhs=xn[:, 2 * tp:2 * tp + 2, n * 512:(n + 1) * 512],
                                start=(tp == 0), stop=(tp == 1), perf_mode=DR)
                    nc.scalar.activation(t1[:, m, :], wt,
                                         mybir.ActivationFunctionType.Identity,
                                         bias=t1b_sb[:, m:m + 1], scale=1.0 / WSCALE)
                vpT = vppool.tile([128, PT, C], FP8, tag="vpT")
                for pp in range(PT // 2):
                    wt = wps.tile([128, 2 * 512], F32, tag="mm")
                    for i in range(2):
                        p = 2 * pp + i
                        for tp in range(2):
                            nc.tensor.matmul(
                                wt[:, i * 512:(i + 1) * 512],
                                lhsT=xn[:, 2 * tp:2 * tp + 2, p * 128:(p + 1) * 128],
                                rhs=h_sb[:, 2 * tp:2 * tp + 2, :],
                                start=(tp == 0), stop=(tp == 1), perf_mode=DR)
                    nc.scalar.activation(vpT[:, 2 * pp:2 * pp + 2, :],
                                         wt.rearrange("p (a b) -> p a b", a=2),
                                         mybir.ActivationFunctionType.Copy,
                                         scale=1.0 / WSCALE)
                return t1, vpT

            def part2a(bb, xn, t1):
                """transposed scores -> exp (fp8) -> colsum -> recip.

                """
                expT = expool.tile([128, PT, HW], FP8, tag="expT")
                for jm in range(PT):
                    wt = wps.tile([128, 2 * 512], F32, tag="mm")
                    for n in range(NB):
                        for tp in range(2):
                            nc.tensor.matmul(
                                wt[:, n * 512:(n + 1) * 512],
                                lhsT=xn[:, 2 * tp:2 * tp + 2, jm * 128:(jm + 1) * 128],
                                rhs=t1[:, 2 * tp:2 * tp + 2, n * 512:(n + 1) * 512],
                                start=(tp == 0), stop=(tp == 1), perf_mode=DR)
                    nc.scalar.activation(expT[:, jm, :], wt,
                                         mybir.ActivationFunctionType.Exp,
                                         bias=shift_sb, scale=SCALE)
                colp = wps.tile([128, 2 * 512], F32, tag="mm")
                for n in range(NB):
                    for jp in range(PT // 2):
                        nc.tensor.matmul(
                            colp[:, n * 512:(n + 1) * 512],
                            lhsT=ones8,
                            rhs=expT[:, 2 * jp:2 * jp + 2, n * 512:(n + 1) * 512],
                            start=(jp == 0), stop=(jp == PT // 2 - 1), perf_mode=DR)
                # recip = exp(-ln(colsum)) on ACT: ln and exp share the
                # resident table set, and the exact DVE reciprocal (6.5us for
                # [128,1024]) would gate the av-stage PSUM rotation.
                lncs = rpool.tile([128, HW], F32, tag="lncs")
                nc.scalar.activation(lncs, colp,
                                     mybir.ActivationFunctionType.Ln)
                recip = rpool.tile([128, HW], F32, tag="recip")
                nc.scalar.activation(recip, lncs,
                                     mybir.ActivationFunctionType.Exp,
                                     scale=-1.0)
                return expT, recip

            def part2b(bb, vpT, expT, recip):
                """res = v' e, normalize, bias, residual, store."""
                xt = xts.pop(bb)
                # Drain every av psum to SBUF via a cheap ACT copy right away:
                # the psum slot frees in ~1.1us regardless of where the DVE is
                # in its queue, so the next batch's t1/scores rotation never
                # starves on a psum bank held hostage by a late normalize.
                resT = respool.tile([128, CT, HW], F32, tag="resT")
                for m in range(CT):
                    wt = wps.tile([128, 2 * 512], F32, tag="mm")
                    for n in range(NB):
                        for jp in range(PT // 2):
                            nc.tensor.matmul(
                                wt[:, n * 512:(n + 1) * 512],
                                lhsT=vpT[:, 2 * jp:2 * jp + 2, m * 128:(m + 1) * 128],
                                rhs=expT[:, 2 * jp:2 * jp + 2, n * 512:(n + 1) * 512],
                                start=(jp == 0), stop=(jp == PT // 2 - 1), perf_mode=DR)
                    nc.scalar.activation(resT[:, m, :], wt,
                                         mybir.ActivationFunctionType.Copy)
                for m in range(CT):
                    tmp = ftpool.tile([128, HW], F32, tag="ft")
                    nc.vector.tensor_mul(tmp, resT[:, m, :], recip)
                    nc.vector.scalar_tensor_tensor(
                        out=xt[:, m, :], in0=tmp, scalar=outb_sb[:, m:m + 1],
                        in1=xt[:, m, :], op0=ADD, op1=ADD)
                    nc.sync.dma_start(
                        out=out_d[bb, m * 128:(m + 1) * 128, :],
                        in_=xt[:, m, :])

            # ---- software pipeline over batches ----
            # Issue order matters for the per-engine FIFOs: norm(bb+1) comes
            # after part2a(bb) so the softmax Exps aren't stuck behind the
            # stats Ln in the ACT queue, and before part2b(bb) so xn(bb+1) is
            # ready the moment the PE finishes av(bb).
            stv_cur = norm_stats(0, norm_bn(0))
            xn_cur = norm_apply(0, stv_cur)
            s3rs = {}
            if B_PER_CORE > 1:
                load_x(1)
                s3rs[1] = norm_bn(1)
            for bb in range(B_PER_CORE):
                t1, vpT = part1(bb, xn_cur)
                expT, recip = part2a(bb, xn_cur, t1)
                if bb + 1 < B_PER_CORE:
                    stv_next = norm_stats(bb + 1, s3rs.pop(bb + 1))
                    xn_next = norm_apply(bb + 1, stv_next)
                else:
                    xn_next = None
                part2b(bb, vpT, expT, recip)
                if bb + 2 < B_PER_CORE:
                    load_x(bb + 2)
                    s3rs[bb + 2] = norm_bn(bb + 2)
                xn_cur = xn_next
    return nc


_NC_CACHE = None


def _q8(v: np.ndarray, scale: float = 1.0) -> np.ndarray:
    """Quantize to TRN e4m3 (saturating at +-240) after scaling."""
    return np.clip(np.asarray(v, np.float64) * scale, -240.0, 240.0).astype(NP8)


def kernel(x, norm_gamma, norm_beta, qkv_w, qkv_b, out_w, out_b):
    global _NC_CACHE
    if _NC_CACHE is None:
        _NC_CACHE = build_nc()
    nc = _NC_CACHE

    x = np.ascontiguousarray(np.asarray(x, np.float32).reshape(B_TOTAL, C, HW))
    qkv_w = np.asarray(qkv_w, np.float64)
    out_w = np.asarray(out_w, np.float64)
    qkv_b = np.asarray(qkv_b, np.float64)
    wq, wk, wv = qkv_w[:C], qkv_w[C:2 * C], qkv_w[2 * C:]
    bq, bk, bv = qkv_b[:C], qkv_b[C:2 * C], qkv_b[2 * C:]

    g8 = np.ascontiguousarray(_q8(wq.T @ wk, WSCALE))           # [c_in, c_out]
    h8 = np.ascontiguousarray(_q8((out_w @ wv).T, WSCALE))      # [c_in, c_out]
    t1b = np.ascontiguousarray((wk.T @ bq).astype(np.float32))
    outb = np.ascontiguousarray(
        (np.asarray(out_b, np.float64) + out_w @ bv).astype(np.float32))
    gamma = np.ascontiguousarray(np.asarray(norm_gamma, np.float32))
    beta = np.ascontiguousarray(np.asarray(norm_beta, np.float32))
    cidx = np.arange(C)
    # each group = 64 channels; selector averages the 64 per-channel stats
    sel = np.ascontiguousarray((cidx[:, None] // (C // GROUPS) == np.arange(GROUPS)[None, :])
                               .astype(np.float32) / (C // GROUPS))
    selT = np.ascontiguousarray((np.arange(GROUPS)[:, None] == cidx[None, :] // (C // GROUPS))
                                .astype(np.float32))

    shared = {"g8": g8, "h8": h8, "t1b": t1b, "outb": outb,
              "gamma": gamma, "beta": beta, "sel": sel, "selT": selT}
    in_maps = [{"x": x[c * B_PER_CORE:(c + 1) * B_PER_CORE], **shared}
               for c in range(N_CORES)]

    trace = bool(int(os.environ.get("KERNEL_TRACE", "0")))
    res = run_bass_kernel_spmd(nc, in_maps, list(range(N_CORES)), trace=trace)
    if trace and res.exec_time_ns is not None:
        print(f"HW exec time: {res.exec_time_ns} ns")
        print(f"(mean across cores: {res.mean_exec_time_ns} ns, "
              f"max core: {res.max_exec_time_core_id})")

    out = np.concatenate([res.results[c]["out"] for c in range(N_CORES)], axis=0)
    return out.reshape(B_TOTAL, C, 32, 32).astype(np.float32)
